# revision 1
# baseline (speedup 1.0000x reference)
"""LoRA self-attention Trainium2 kernel, 8-way head/tensor parallel.

Per-core engine budget (TimelineSim ~224 us): the ACT exp stream (~133 us;
softmax exp = 16.8M elements/core at 1 elem/cycle/partition @1.2GHz on the
only engine with exp) is the hard floor; the schedule keeps it gap-free
(~7 us of in-stream gaps) by ensuring nothing with unresolved dependencies
enters the in-order PE queue ahead of the energy matmuls.

Sharding: core c owns heads 2c, 2c+1 (= channels 128c..128c+128) for the
QKV projections and attention; the output projection is token-sharded
(core c computes all 1024 output channels for tokens 512c..512c+512)
after an AllToAll exchange of the attention output.

Design:
- LoRA folded into the dense weights on host (W_eff = W + 2*B@A, exact).
- Softmax denominator free via the ones column in the augmented-V layout
  (M=65 AV matmuls: attn@V in psum partitions 0..63, denominator in 64).
- Block normalization (reciprocal/replicate/multiply/ship) is deferred and
  interleaved into the next block with dependency-aware slotting: prev
  av(15) at ik0, next-Q matmuls at ik1/4/5, replication matmuls at ik2/3,
  this block's AVs from ik6 (two per ik) once the previous block's PSUM
  accumulators are free.
- bf16 everywhere upstream (x, QKV weights, Q/K/V, exp(P), V_aug, att,
  Wo): halves DMA traffic, same PE rate, ~0.5% worst-case error vs the
  2e-2 budget.
- x is DMA'd once into SBUF (host pre-arranged [128, (j e t)] layout)
  and stays resident; projections K,V run first (attention can start
  after the first batch's K/V), Q is projected just-in-time per
  attention block.
- V transposes are folded into the projection loop (per j-tile).
- Output projection computed transposed (tokens on partitions): 1024-col
  bf16 moving operands, bias added via a K=1 ones-row matmul, Y written
  [512 tok, 1024 ch] contiguously.
"""
import sys

for p in ("/opt/trn_rl_repo",):
    if p not in sys.path:
        sys.path.append(p)

import numpy as np

import concourse.bass as bass  # noqa: F401
import concourse.tile as tile
from concourse import bacc, mybir
from concourse import bass_utils

N_CORES = 8
EMBED = 1024
HEADS = 16
HD = 64            # head dim
NB = 2             # batch
S = 2048           # seq len
T = NB * S         # 4096 tokens
CH = EMBED // N_CORES  # 128 channels (2 heads) per core
FP = mybir.dt.float32
FPR = mybir.dt.float32r
BF = mybir.dt.bfloat16
AF = mybir.ActivationFunctionType
BF_NP = mybir.dt.np(mybir.dt.bfloat16)

_CACHE: dict = {}

NE = EMBED // 128  # 8 contraction tiles
NJ = T // 512      # 8 token tiles


def _build(local_only=False):
    nc = bacc.Bacc("TRN2", target_bir_lowering=False, debug=False,
                   enable_asserts=False, num_devices=N_CORES)
    # ---- DRAM I/O (per-core) ----
    # x pre-arranged on host: [128, (j e t)] = [128, 8*8*512]
    xP = nc.dram_tensor("xP", [128, NJ * NE * 512], BF, kind="ExternalInput").ap()
    # weights pre-arranged: [128, (e c)] = [128, 8*128]
    # packed weights: [wk | wv | wq] each [128, NE*CH]
    wP = nc.dram_tensor("wP", [128, 3 * NE * CH], BF, kind="ExternalInput").ap()
    ident = nc.dram_tensor("ident", [128, 128], BF, kind="ExternalInput").ap()
    bias3 = nc.dram_tensor("bias3", [CH, 3], FP, kind="ExternalInput").ap()
    woT = nc.dram_tensor("woT", [EMBED, EMBED], BF, kind="ExternalInput").ap()
    # packed row constants: bo[0:1024] | ones1[1024:1152] | selA | selB
    rowP = nc.dram_tensor("rowP", [1, 1408], BF, kind="ExternalInput").ap()
    Y = nc.dram_tensor("Y", [512, EMBED], BF, kind="ExternalOutput").ap()

    with tile.TileContext(nc) as tc, \
         nc.allow_low_precision(reason="bf16 rounding is intentional"):
        with tc.tile_pool(name="const", bufs=1) as cpool, \
             tc.tile_pool(name="big", bufs=1) as bigpool, \
             tc.tile_pool(name="dram", bufs=1, space="DRAM") as dram:

            # ---- resident weights, one packed DMA (queue issue costs
            # ~1.7us per dma_start; fewer, larger transfers start compute
            # sooner) ----
            w_all = cpool.tile([128, 3 * NE * CH], BF, tag="wall")
            nc.sync.dma_start(w_all[:], wP)
            wk_sb = w_all[:, 0:NE * CH]
            wv_sb = w_all[:, NE * CH:2 * NE * CH]
            wq_sb = w_all[:, 2 * NE * CH:3 * NE * CH]
            # x j0 right after the weights so the first matmul starts ASAP
            x_sb = bigpool.tile([128, NJ * NE * 512], BF, tag="x")
            nc.sync.dma_start(x_sb[:, 0:NE * 512], xP[:, 0:NE * 512])
            # constants (needed by the first copies at ~14us)
            id_sb = cpool.tile([128, 128], BF, tag="ident")
            nc.sync.dma_start(id_sb[:], ident)
            bias_sb = cpool.tile([CH, 3], FP, tag="bias3")
            nc.sync.dma_start(bias_sb[:], bias3)
            row_sb = cpool.tile([1, 1408], BF, tag="rowP")
            nc.sync.dma_start(row_sb[:], rowP)
            bo_sb = row_sb[:, 0:1024]
            o1_sb = row_sb[:, 1024:1152]
            selA = row_sb[:, 1152:1280]
            selB = row_sb[:, 1280:1408]
            # ---- rest of x (per-j chunks) ----
            for j in range(1, NJ):
                sl = slice(j * NE * 512, (j + 1) * NE * 512)
                nc.sync.dma_start(x_sb[:, sl], xP[:, sl])
            # phase-D weights: DMA'd up front, hidden under compute
            wo_sb = [cpool.tile([128, EMBED], BF, tag=f"wo{ci}", name=f"wo{ci}")
                     for ci in range(NE)]
            for ci in range(NE):
                nc.sync.dma_start(wo_sb[ci][:], woT[ci * 128:(ci + 1) * 128, :])

            # ---- resident activations ----
            QT_sb = bigpool.tile([CH, T], BF, tag="QT")
            KT_sb = bigpool.tile([CH, T], BF, tag="KT")
            VT_sb = bigpool.tile([CH, T], BF, tag="VT")
            # V in [token, ch] layout, 32 strips of [128, 130]:
            # cols [s*130+h*65 : +64] = V head h, col [s*130+h*65+64] = ones
            V_sb = bigpool.tile([128, 32 * 130], BF, tag="Vaug")

            def xt(j, e):
                return x_sb[:, (j * NE + e) * 512:(j * NE + e + 1) * 512]

            # ones columns of the augmented-V layout (all strips, once)
            v_ones = V_sb.rearrange("p (s c) -> p s c", c=65)[:, :, 64]
            nc.vector.memset(v_ones, 1.0)

            # ========== Phase A: K,V projections + V transpose ==========
            # PSUM evictions go to DVE so the ACT engine stays free for the
            # exp stream.
            with tc.tile_pool(name="psA", bufs=2, space="PSUM") as psA, \
                 tc.tile_pool(name="psT", bufs=2, space="PSUM") as psT:
                for j in range(NJ):
                    t0 = j * 512
                    pk = psA.tile([CH, 512], FP, tag="k", name="pk")
                    pv = psA.tile([CH, 512], FP, tag="v", name="pv")
                    for e in range(NE):
                        nc.tensor.matmul(pv[:], wv_sb[:, e * CH:(e + 1) * CH],
                                         xt(j, e), start=(e == 0),
                                         stop=(e == NE - 1))
                        nc.tensor.matmul(pk[:], wk_sb[:, e * CH:(e + 1) * CH],
                                         xt(j, e), start=(e == 0),
                                         stop=(e == NE - 1))
                    nc.vector.tensor_scalar_add(VT_sb[:, t0:t0 + 512],
                                                pv[:], bias_sb[:, 2:3])
                    nc.vector.tensor_scalar_add(KT_sb[:, t0:t0 + 512],
                                                pk[:], bias_sb[:, 1:2])
                    # transpose this j-tile's V into the augmented layout
                    for si in range(4):
                        t = j * 4 + si
                        trp = psT.tile([128, 128], BF, tag="tr", name="trp")
                        nc.tensor.transpose(trp[:],
                                            VT_sb[:, t * 128:(t + 1) * 128],
                                            id_sb[:])
                        base = t * 130
                        nc.vector.tensor_copy(V_sb[:, base:base + 64],
                                              trp[:, 0:64])
                        nc.vector.tensor_copy(V_sb[:, base + 65:base + 129],
                                              trp[:, 64:128])

            # ========== Phase C: attention (Q projected just-in-time) ======
            bounce_in = dram.tile([N_CORES, 128, 512], BF)
            bounce_out = dram.tile([N_CORES, 128, 512], BF)
            with tc.tile_pool(name="psC", bufs=2, space="PSUM") as psC, \
                 tc.tile_pool(name="psO", bufs=1, space="PSUM") as psO, \
                 tc.tile_pool(name="psQ", bufs=1, space="PSUM") as psQ, \
                 tc.tile_pool(name="psR", bufs=1, space="PSUM") as psR, \
                 tc.tile_pool(name="pt", bufs=10) as ptpool, \
                 tc.tile_pool(name="rs", bufs=2) as rpool:
                qtiles = {}

                def qproj_mm(j, e):
                    pq = qtiles[j]
                    nc.tensor.matmul(pq[:], wq_sb[:, e * CH:(e + 1) * CH],
                                     xt(j, e), start=(e == 0),
                                     stop=(e == NE - 1))
                    if e == NE - 1:
                        q0j = (j // 4) * S + (j % 4) * 512
                        nc.vector.tensor_scalar_add(
                            QT_sb[:, q0j:q0j + 512], pq[:], bias_sb[:, 0:1])

                qtiles[0] = psQ.tile([CH, 512], FP, tag="q", name="pq")
                for e in range(NE):
                    qproj_mm(0, e)

                pending = None  # previous block's deferred tail
                for n in range(NB):
                    for jq in range(4):
                        j = n * 4 + jq
                        q0 = n * S + jq * 512
                        po0 = psO.tile([65, 512], FP, tag="o0", name="po0")
                        po1 = psO.tile([65, 512], FP, tag="o1", name="po1")
                        pts = {}
                        first = pending is None

                        def av(ik, n=n, po0=po0, po1=po1, pts=pts):
                            # attn@V plus denominator (ones column) in one
                            # M=65 matmul per head
                            vb = (n * 16 + ik) * 130
                            ptp = pts.pop(ik)
                            nc.tensor.matmul(
                                po0[0:65, :], V_sb[:, vb:vb + 65],
                                ptp[:, 0:512],
                                start=(ik == 0), stop=(ik == 15))
                            nc.tensor.matmul(
                                po1[0:65, :], V_sb[:, vb + 65:vb + 130],
                                ptp[:, 512:1024],
                                start=(ik == 0), stop=(ik == 15))

                        # per-ik PE fillers between the two K=64 energy
                        # matmuls (avoids adjacent row-tiled matmuls) chosen
                        # so nothing enters the in-order PE queue before its
                        # dependencies are met:
                        #   first block: previous-strip AV (no WAR pressure)
                        #   later blocks: ik0 = prev av(15); ik1/4/5 = next-Q
                        #   matmuls; ik2/3 = prev rep matmuls (recips ready);
                        #   avs of THIS block start at ik6 (prev muls done),
                        #   two per ik to catch up.
                        av_next = 0
                        qe = {}
                        if j < 7:
                            qe = {1: 0, 4: 1, 5: 2}
                        for ik in range(16):
                            k0 = n * S + ik * 128
                            pe = psC.tile([128, 1024], FP, tag="pe", name="pe")
                            nc.tensor.matmul(
                                pe[:, 0:512],
                                KT_sb[0:HD, k0:k0 + 128],
                                QT_sb[0:HD, q0:q0 + 512],
                                start=True, stop=True)
                            # one 128-row filler between the energy pair
                            if first:
                                if ik > 0:
                                    av(ik - 1)
                                    av_next = ik
                            elif ik == 0:
                                pending["av15"]()
                            elif ik == 1 and j < 7:
                                qtiles[j + 1] = psQ.tile(
                                    [CH, 512], FP, tag="q", name="pq")
                                qproj_mm(j + 1, 0)
                            elif ik == 2:
                                nc.tensor.matmul(pending["rep2"][:], selA,
                                                 pending["rd0"][:],
                                                 start=True, stop=False)
                            elif ik == 3:
                                nc.tensor.matmul(pending["rep2"][:], selB,
                                                 pending["rd1"][:],
                                                 start=False, stop=True)
                            elif ik in qe:
                                qproj_mm(j + 1, qe[ik])
                            elif ik >= 6 and av_next < ik:
                                av(av_next)
                                av_next += 1
                            nc.tensor.matmul(
                                pe[:, 512:1024],
                                KT_sb[HD:128, k0:k0 + 128],
                                QT_sb[HD:128, q0:q0 + 512],
                                start=True, stop=True)
                            pt = ptpool.tile([128, 1024], BF, tag="pt",
                                             name="pt")
                            nc.scalar.activation(pt[:], pe[:], AF.Exp,
                                                 scale=0.125)
                            pts[ik] = pt
                            # post-exp work (off the exp critical path)
                            if not first:
                                if ik == 0:
                                    # reciprocals of the previous block's
                                    # denominators (DVE)
                                    nc.vector.reciprocal(
                                        pending["rd0"][:],
                                        pending["po0"][64:65, :])
                                    nc.vector.reciprocal(
                                        pending["rd1"][:],
                                        pending["po1"][64:65, :])
                                elif ik == 3:
                                    # replicate -> multiply -> ship (Pool/DVE)
                                    rep = rpool.tile([128, 512], FP,
                                                     tag="repS", name="rep")
                                    nc.vector.tensor_copy(
                                        rep[:], pending["rep2"][:])
                                    attb = rpool.tile([128, 512], BF,
                                                      tag="attb", name="attb")
                                    nc.vector.tensor_mul(
                                        attb[0:64, :],
                                        pending["po0"][0:64, :], rep[0:64, :])
                                    nc.vector.tensor_mul(
                                        attb[64:128, :],
                                        pending["po1"][0:64, :],
                                        rep[64:128, :])
                                    nc.sync.dma_start(
                                        bounce_in[pending["j"]], attb[:])
                            if not first and ik >= 6 and av_next < ik:
                                # second AV this ik while catching up
                                av(av_next)
                                av_next += 1
                            if first and j < 7 and ik >= 8:
                                if ik == 8:
                                    qtiles[j + 1] = psQ.tile(
                                        [CH, 512], FP, tag="q", name="pq")
                                qproj_mm(j + 1, ik - 8)
                            elif not first and j < 7 and 6 <= ik < 11:
                                qproj_mm(j + 1, ik - 3)
                        pending = {
                            "av15": (lambda av=av: av(15)),
                            "po0": po0, "po1": po1, "j": j,
                            "rd0": rpool.tile([1, 512], BF, tag="rd0",
                                              name="rd0"),
                            "rd1": rpool.tile([1, 512], BF, tag="rd1",
                                              name="rd1"),
                            "rep2": psR.tile([128, 512], FP, tag="rep",
                                             name="rep2"),
                        }
                # final block tail
                pending["av15"]()
                nc.vector.reciprocal(pending["rd0"][:], pending["po0"][64:65, :])
                nc.vector.reciprocal(pending["rd1"][:], pending["po1"][64:65, :])
                nc.tensor.matmul(pending["rep2"][:], selA, pending["rd0"][:],
                                 start=True, stop=False)
                nc.tensor.matmul(pending["rep2"][:], selB, pending["rd1"][:],
                                 start=False, stop=True)
                rep = rpool.tile([128, 512], FP, tag="repS", name="rep")
                nc.vector.tensor_copy(rep[:], pending["rep2"][:])
                attb = rpool.tile([128, 512], BF, tag="attb", name="attb")
                nc.vector.tensor_mul(attb[0:64, :], pending["po0"][0:64, :],
                                     rep[0:64, :])
                nc.vector.tensor_mul(attb[64:128, :], pending["po1"][0:64, :],
                                     rep[64:128, :])
                nc.sync.dma_start(bounce_in[pending["j"]], attb[:])

            # ================= AllToAll redistribution =================
            if local_only:
                nc.sync.dma_start(bounce_out[:], bounce_in[:])
            else:
                nc.gpsimd.collective_compute(
                    "AllToAll", mybir.AluOpType.bypass,
                    ins=[bounce_in.opt()], outs=[bounce_out.opt()],
                    replica_groups=[list(range(N_CORES))],
                )

            # ====== Phase D: output projection, transposed (tok on part) ====
            with tc.tile_pool(name="psD", bufs=2, space="PSUM") as psD, \
                 tc.tile_pool(name="dsb", bufs=1) as dpool, \
                 tc.tile_pool(name="ybuf", bufs=2) as ypool:
                att_sb = []
                for i in range(NE):
                    a = dpool.tile([128, 512], BF, tag=f"att{i}", name=f"att{i}")
                    nc.sync.dma_start(a[:], bounce_out[i])
                    att_sb.append(a)
                for tk in range(4):
                    y_sb = ypool.tile([128, EMBED], BF, tag="y_sb")
                    py0 = psD.tile([128, 512], FP, tag="y0")
                    py1 = psD.tile([128, 512], FP, tag="y1")
                    # ci outer / half inner: one att LDWEIGHTS serves both
                    # 512-col output halves
                    for ci in range(NE):
                        nc.tensor.matmul(
                            py0[:], att_sb[ci][:, tk * 128:(tk + 1) * 128],
                            wo_sb[ci][:, 0:512], start=(ci == 0), stop=False)
                        nc.tensor.matmul(
                            py1[:], att_sb[ci][:, tk * 128:(tk + 1) * 128],
                            wo_sb[ci][:, 512:1024], start=(ci == 0), stop=False)
                    # bias: K=1 ones-row x bo_row
                    nc.tensor.matmul(py0[:], o1_sb, bo_sb[:, 0:512],
                                     start=False, stop=True)
                    nc.tensor.matmul(py1[:], o1_sb, bo_sb[:, 512:1024],
                                     start=False, stop=True)
                    nc.scalar.copy(y_sb[:, 0:512], py0[:])
                    nc.scalar.copy(y_sb[:, 512:1024], py1[:])
                    nc.sync.dma_start(Y[tk * 128:(tk + 1) * 128, :], y_sb[:])
    nc.compile()
    return nc


def _prep_inputs(x, Wq, bq, Aq, Bq, Wk, bk, Ak, Bk, Wv, bv, Av, Bv, Wo, bo, Ao, Bo):
    f32 = np.float32
    f64 = np.float64
    xT = x.reshape(T, EMBED).T.astype(f32)          # [1024, 4096]
    # [128, (j e t)]: row p, col ((j*8+e)*512 + t) = xT[e*128+p, j*512+t]
    xPm = np.ascontiguousarray(
        xT.reshape(NE, 128, NJ, 512).transpose(1, 2, 0, 3).reshape(128, -1)
    ).astype(BF_NP)
    # fold LoRA into the dense weights (exact algebra)
    Wq_eff = (Wq.astype(f64) + 2.0 * Bq.astype(f64) @ Aq.astype(f64)).astype(f32)
    Wk_eff = (Wk.astype(f64) + 2.0 * Bk.astype(f64) @ Ak.astype(f64)).astype(f32)
    Wv_eff = (Wv.astype(f64) + 2.0 * Bv.astype(f64) @ Av.astype(f64)).astype(f32)
    Wo_eff = (Wo.astype(f64) + 2.0 * Bo.astype(f64) @ Ao.astype(f64)).astype(f32)

    def wprep(Weff, sl):
        # W.T slice [1024, 128] -> [128, (e c)]
        wT = Weff[sl, :].T.astype(f32)              # [1024, 128]
        return np.ascontiguousarray(
            wT.reshape(NE, 128, CH).transpose(1, 0, 2).reshape(128, -1)
        ).astype(BF_NP)

    identm = np.eye(128, dtype=f32).astype(BF_NP)
    sel2m = np.zeros((2, 128), dtype=f32)
    sel2m[0, 0:64] = 1.0
    sel2m[1, 64:128] = 1.0
    sel2m = sel2m.astype(BF_NP)
    woTm = np.ascontiguousarray(Wo_eff.T.astype(BF_NP))
    in_maps = []
    for c in range(N_CORES):
        sl = slice(c * CH, (c + 1) * CH)
        bias3m = np.stack([bq[sl], bk[sl], bv[sl]], axis=1).astype(f32)
        wPm = np.concatenate([wprep(Wk_eff, sl), wprep(Wv_eff, sl),
                              wprep(Wq_eff, sl)], axis=1)

        rowPm = np.concatenate(
            [bo.reshape(1, EMBED).astype(BF_NP),
             np.ones((1, 128), dtype=f32).astype(BF_NP),
             sel2m[0:1], sel2m[1:2]], axis=1)
        in_maps.append({
            "xP": xPm,
            "wP": np.ascontiguousarray(wPm),
            "ident": identm,
            "bias3": np.ascontiguousarray(bias3m),
            "woT": woTm,
            "rowP": np.ascontiguousarray(rowPm),
        })
    return in_maps


def get_nc():
    if "nc" not in _CACHE:
        _CACHE["nc"] = _build()
    return _CACHE["nc"]


def kernel(**inputs) -> np.ndarray:
    nc = get_nc()
    in_maps = _prep_inputs(**{k: np.asarray(v) for k, v in inputs.items()})
    res = bass_utils.run_bass_kernel_spmd(
        nc, in_maps, core_ids=list(range(N_CORES)))
    y = np.concatenate([np.asarray(res.results[c]["Y"], dtype=np.float32)
                        for c in range(N_CORES)], axis=0)
    return np.ascontiguousarray(y).reshape(NB, S, EMBED)


if __name__ == "__main__":
    nc = get_nc()
    print("build+compile OK")

